# revision 22
# baseline (speedup 1.0000x reference)
"""Single-head attention (shared-input QKV projections) on 8 Trainium2 cores.

Reference computation (per batch b):
    q = x[b] @ Wq; k = x[b] @ Wk; v = x[b] @ Wv        # [S, 64]
    out[b] = softmax(q @ k.T / 8) @ v                  # [S, 64]
with B=4, S=4096, D=256, OUT=64.

Sharding: data-parallel over batch (4 batches x 2 cores) with
sequence-parallel query halves; per-core query offset handled by host-side
row rotation of x[b] (attention is permutation-invariant over key rows).

Per-core kernel (v2 - fp8 attention pipeline):
  - Projections in f32r (TF32-class). K and V share one stationary
    ([d-half, k|v]) so one matmul pair produces both; k lands on
    partitions 0:64, v on 64:128 of one PSUM tile -> single copy.
  - Scores computed transposed S^T[k, q] in f32r, one [128, W] chunk per
    matmul (contraction 64).
  - exp via the Schraudolph int8 bit trick *directly into fp8e4 bits*:
      i8 = rint(s_raw * log2e + BC); bitcast(i8) ~= exp(s_raw/8) * 2^(BC/8-7)
    emitted on ACT (Identity activation, scale+bias) and DVE
    (tensor_scalar) alternately - both engines round-to-nearest so the
    values are bit-identical across engines. No max-subtraction (score
    range is bounded; i8 stays in [~10, ~115]).
  - attn @ V as fp8 DoubleRow matmuls: one matmul per k-chunk PAIR
    (stationary [128, 2, 65] = two V chunks each with an appended ones
    column; moving [128, 2, W] = the two exp chunks) at 0.5 cycles/col -
    4x the f32r rate. The ones column accumulates the softmax denominator.
  - Output is the UNNORMALIZED [65, W] numerator+denominator per q-block;
    the host does the final divide (free).
V chunks are built by PE transposes (4 per PSUM tile, start=False
accumulate trick) + one batched DVE copy to fp8.
"""

import numpy as np

import concourse.mybir as mybir
import concourse.tile as tile
from concourse import bacc
from concourse.masks import make_identity

P = 128
D = 256
OUT = 64
SCALE = 0.125
F32 = mybir.dt.float32
F32R = mybir.dt.float32r
FP8 = mybir.dt.float8e4
I8 = mybir.dt.int8

B_FULL, S_FULL = 4, 4096
N_CORES = 8

LOG2E = 1.4426950408889634
BC = 59.7  # schraudolph bias: i8 = rint(s_raw*log2e + BC)



def build_nc(S: int, QH: int, QB_W: int = 512, loop_n: int | None = None,
             timing_mode: bool = False, lag: int = 4, no_dma: bool = False,
             unroll: int = 1):
    """Build the per-core SPMD program.

    S: sequence length (key/value rows) held by this core.
    QH: number of query rows this core computes (first QH rows of x).
    QB_W: query block width (free dim of the score matmuls).
    loop_n: if set, run the whole body loop_n times on device (for timing).
    timing_mode: shrink the xt input to 512 cols (reread on device) so
        host->device transfer noise doesn't swamp loop-delta timing.
    """
    assert S % 512 == 0 and QH % QB_W == 0 and QB_W % P == 0
    nc = bacc.Bacc()
    xt_cols = 512 if timing_mode else S
    xt_in = nc.declare_dram_parameter("xt", [2, P, xt_cols], F32R, isOutput=False)
    w_in = nc.declare_dram_parameter("w", [2, P, 192], F32R, isOutput=False)
    nqb = QH // QB_W
    out_d = nc.declare_dram_parameter("out", [nqb, 65, QB_W], F32, isOutput=True)

    with tile.TileContext(nc) as tc:
        with (
            tc.tile_pool(name="const", bufs=1) as constp,
            tc.tile_pool(name="big", bufs=2) as bigp,
            tc.tile_pool(name="attnp", bufs=8) as attnp,
            tc.tile_pool(name="obuf", bufs=2) as obufp,
            tc.tile_pool(name="stps", bufs=3, space="PSUM") as stps,
            tc.tile_pool(name="psps", bufs=1, space="PSUM") as psps,
            tc.tile_pool(name="pops", bufs=1, space="PSUM") as pops,
        ):
            ident = constp.tile([P, P], F32)
            make_identity(nc, ident)
            identr = constp.tile([P, P], F32R)
            nc.vector.tensor_copy(identr, ident)
            bias_t = constp.tile([P, 1], F32)
            nc.vector.memset(bias_t, float(BC))
            w_sb = constp.tile([P, 2, 192], F32R)
            for h in range(2):
                nc.sync.dma_start(w_sb[:, h, :], w_in[h, :, :])
            if loop_n is not None:
                # Unrolled bodies inside the hw loop: tile pools (bufs=2)
                # alternate buffers across the unrolled copies, so body u+1's
                # DMA + projections overlap body u's attention drain; the
                # loop-edge barrier cost is amortized over `unroll` bodies.
                assert loop_n % unroll == 0
                loop_cm = tc.For_i(0, loop_n // unroll, 1)
                loop_cm.__enter__()
                for _ in range(unroll):
                    _emit_body(nc, tc, xt_in, out_d, S, QH, QB_W, lag, constp,
                               bigp, attnp, obufp, stps, psps, pops, identr,
                               bias_t, w_sb, timing_mode, no_dma)
                loop_cm.__exit__(None, None, None)
            else:
                _emit_body(nc, tc, xt_in, out_d, S, QH, QB_W, lag, constp,
                           bigp, attnp, obufp, stps, psps, pops, identr,
                           bias_t, w_sb, timing_mode, no_dma)
    return nc


def _emit_body(nc, tc, xt_in, out_d, S, QH, QB_W, lag, constp, bigp, attnp,
               obufp, stps, psps, pops, identr, bias_t, w_sb,
               timing_mode=False, no_dma=False):
    nk = S // P          # 128-row k chunks
    npair = nk // 2      # chunk pairs (one DoubleRow attn@V matmul each)
    ngrp = S // 512      # 512-col production groups (kv proj + 4 v chunks)
    nqb = QH // QB_W

    # cost-tracking balancer: assign each elementwise op to the engine
    # (ACT/DVE) with less accumulated busy-time. Schraudolph needs no act
    # table, so ACT can take copies freely (Copy is table-free too).
    acc = {"A": 0.0, "D": 0.0}

    def pick(cost_a, cost_d):
        e = "A" if acc["A"] + cost_a <= acc["D"] + cost_d else "D"
        acc[e] += cost_a if e == "A" else cost_d
        return e

    def bal_copy(dst, src_ap, free):
        if pick((free + 222) * 0.833 + 300, (free + 120) * 1.042 + 330) == "A":
            nc.scalar.copy(dst, src_ap)
        else:
            nc.vector.tensor_copy(dst, src_ap)

    xt = bigp.tile([P, 2, S], F32R)
    kv = bigp.tile([P, S], F32R)     # k on partitions 0:64, v on 64:128
    qt = bigp.tile([64, QH], F32R)
    v8 = bigp.tile([P, nk, P], FP8)  # chunk kc at [:, kc, 0:65]

    if no_dma:
        nc.sync.dma_start(xt[:, 0, 0:1], xt_in[0, :, 0:1])  # minimal read
    # x^T slices; the two d-halves ride different DMA queues (SP / gpsimd).
    for h in range(2) if not no_dma else []:
        eng = nc.sync if h == 0 else nc.gpsimd
        lo = 0
        for wdt in ([512] * (S // 512) if timing_mode else
                    [512, 512] + [1024] * ((S - 1024) // 1024)):
            src_lo = 0 if timing_mode else lo
            eng.dma_start(xt[:, h, lo:lo + wdt],
                          xt_in[h, :, src_lo:src_lo + wdt])
            lo += wdt
    # softmax-denominator ones column (fp8 1.0)
    nc.gpsimd.memset(v8[:, :, 64], 1.0)

    def qproj(qb, sub):
        lo = qb * QB_W + sub * 512
        pp = psps.tile([P, 512], F32, name="pp", tag="mps")
        for h in range(2):
            nc.tensor.matmul(pp[0:64, :], w_sb[:, h, 0:64],
                             xt[:, h, lo:lo + 512],
                             start=(h == 0), stop=(h == 1))
        bal_copy(qt[:, lo:lo + 512], pp[0:64, :], 512)

    def kvproj(g):
        lo = g * 512
        pp = psps.tile([P, 512], F32, name="pp", tag="mps")
        for h in range(2):
            nc.tensor.matmul(pp, w_sb[:, h, 64:192],
                             xt[:, h, lo:lo + 512],
                             start=(h == 0), stop=(h == 1))
        bal_copy(kv[:, lo:lo + 512], pp, 512)

    def vchunks(g):
        """Transpose the 4 V chunks of group g into v8 (one batched copy)."""
        tv4 = psps.tile([P, 512], F32, name="pp", tag="mps")
        for j in range(4):
            kc = 4 * g + j
            nc.tensor.matmul(
                tv4[:, j * 64:(j + 1) * 64].bitcast(F32R),
                kv[64:128, kc * P:(kc + 1) * P],
                identr[64:128, 64:128],
                start=(j == 0), stop=(j == 3),
                is_transpose=True, skip_group_check=True,
            )
        bal_copy(v8[:, 4 * g:4 * g + 4, 0:64],
                 tv4[:, 0:256].bitcast(F32R).rearrange("p (f c) -> p f c",
                                                       f=4), 256)

    def pair_part(qb, t):
        """Score matmuls for pair t into one PSUM tile + one pair-wide exp."""
        qs = qb * QB_W
        st = stps.tile([P, 2 * QB_W], F32, name="st", tag="st")
        for j in range(2):
            kc = 2 * t + j
            nc.tensor.matmul(
                st[:, j * QB_W:(j + 1) * QB_W],
                kv[0:64, kc * P:(kc + 1) * P],
                qt[:, qs:qs + QB_W],
                start=True, stop=True,
            )
        at = attnp.tile([P, 2 * QB_W], FP8, name="at", tag="at")
        ati = at[:, :].bitcast(I8)
        w = 2 * QB_W
        if pick((w + 222) * 0.833 + 300, (w + 120) * 1.042 + 330) == "A":
            nc.scalar.activation(ati, st,
                                 mybir.ActivationFunctionType.Identity,
                                 scale=float(LOG2E), bias=bias_t[:, 0:1])
        else:
            nc.vector.tensor_scalar(ati, st, float(LOG2E), float(BC),
                                    mybir.AluOpType.mult, mybir.AluOpType.add)
        return at

    po_tiles = {}

    def av_part(qb, t, at):
        if t == 0:
            po_tiles[qb] = pops.tile([65, QB_W], F32, name="po", tag="po")
        nc.tensor.matmul(
            po_tiles[qb],
            v8[:, 2 * t:2 * t + 2, 0:65],
            at[:, :].rearrange("p (two w) -> p two w", two=2),
            start=(t == 0), stop=(t == npair - 1),
            perf_mode=mybir.MatmulPerfMode.DoubleRow,
        )

    def epilogue(qb):
        po = po_tiles.pop(qb)
        ob = obufp.tile([65, QB_W], F32, name="ob", tag="ob")
        bal_copy(ob, po, QB_W)
        nc.sync.dma_start(out_d[qb, :, :], ob)

    # --- schedule ---
    # Prime just enough for the first two pairs; spread the remaining
    # production through the pair stream with a ~2-group lead so the exp
    # engines start immediately and never starve.
    nqsub = QB_W // 512
    for sub in range(nqsub):
        qproj(0, sub)
    kvproj(0)
    vchunks(0)
    kvproj(1)

    pairs = [(qb, t) for qb in range(nqb) for t in range(npair)]
    prod = {0: [lambda: vchunks(1)]}
    for g in range(2, ngrp):
        prod.setdefault(max(0, 2 * g - 4), []).append(lambda g=g: kvproj(g))
        prod.setdefault(max(1, 2 * g - 3), []).append(lambda g=g: vchunks(g))
    for qb in range(1, nqb):
        for sub in range(nqsub):
            prod.setdefault(npair * qb - 5 + sub, []).append(
                lambda qb=qb, sub=sub: qproj(qb, sub))

    pend = []
    for i, (qb, t) in enumerate(pairs):
        for task in prod.get(i, []):
            task()
        at = pair_part(qb, t)
        pend.append((qb, t, at))
        if len(pend) > lag:
            qb0, t0, at0 = pend.pop(0)
            av_part(qb0, t0, at0)
            if t0 == npair - 1:
                epilogue(qb0)
    for qb0, t0, at0 in pend:
        av_part(qb0, t0, at0)
        if t0 == npair - 1:
            epilogue(qb0)


_compiled_nc = None
LAST_RESULT = None  # BassKernelResults of the most recent kernel() call


def _get_compiled_nc():
    global _compiled_nc
    if _compiled_nc is None:
        nc = build_nc(S_FULL, S_FULL // 2)
        nc.compile()
        _compiled_nc = nc
    return _compiled_nc


def stage_w(w):
    """[3, 256, 64] -> [2, 128, 192] f32 (d-half, partition, q|k|v)."""
    full = np.concatenate([w[0], w[1], w[2]], axis=1)  # [256, 192]
    return np.ascontiguousarray(full.reshape(2, P, 192).astype(np.float32))


def make_in_maps(x, w):
    """Host-side staging: roll per query half, transpose to d-major."""
    qh = S_FULL // 2
    ws = stage_w(w)
    in_maps = []
    for c in range(N_CORES):
        b, h = c // 2, c % 2
        xb = x[b]
        xr = xb if h == 0 else np.concatenate([xb[qh:], xb[:qh]], axis=0)
        xtc = np.ascontiguousarray(xr.T).reshape(2, P, S_FULL)
        in_maps.append({"xt": xtc, "w": ws})
    return in_maps


def kernel(x, kernel):
    from concourse.bass_utils import run_bass_kernel_spmd

    x = np.asarray(x, dtype=np.float32)
    w = np.asarray(kernel, dtype=np.float32)
    assert x.shape == (B_FULL, S_FULL, D) and w.shape == (3, D, OUT)
    qh = S_FULL // 2

    nc = _get_compiled_nc()
    res = run_bass_kernel_spmd(nc, make_in_maps(x, w),
                               core_ids=list(range(N_CORES)))
    global LAST_RESULT
    LAST_RESULT = res
    out = np.empty((B_FULL, S_FULL, OUT), dtype=np.float32)
    for c in range(N_CORES):
        b, h = c // 2, c % 2
        r = res.results[c]["out"]          # [nqb, 65, QB_W]
        num = r[:, :64, :]
        den = r[:, 64, :]
        half = np.transpose(num / den[:, None, :], (0, 2, 1)).reshape(qh, OUT)
        out[b, h * qh:(h + 1) * qh] = half
    return out
